# revision 1
# baseline (speedup 1.0000x reference)
"""Trainium2 Bass kernel for the AttnBlock-style attention module.

Reference computation (note softmax over axis=1, the *i* axis):
    q = wq @ x + bq ; k = wk @ x + bk ; v = wv @ x + bv      (per-pixel 1x1 conv)
    s[b,i,j] = (q[b,:,i] . k[b,:,j]) * C**-0.5
    attn = softmax_i(s)                                      (normalize over i!)
    out[b,c,i] = sum_j attn[b,i,j] v[b,c,j]
    y = wp @ out + bp

Sharding: 8 cores = 4 batches x 2 j-halves (see baseline docstring).

Scheduling changes vs the 155.3us baseline (measured ~148-149us):
  - x / weights / biases host-packed into SBUF-layouts so DMAs move 2-4KB
    contiguous runs per partition; wk is shipped first (small DMA) so the
    first k-projection can start as early as possible; block DMAs alternate
    between the SP and ACT hardware DGE queues.
  - warmup tile lives in the const pool and the dummy exp reads SBUF, so no
    WAR dependency can delay the x-block DMAs.
  - early score tiles are interleaved between projection groups instead of
    emitted in a block (ACT exp is slower than the PE per tile; consecutive
    s_tiles head-of-line block the PE queue on PSUM recycling).
  - y is produced in bf16 (host upcasts/sums), final adds write bf16 and the
    8 output stores alternate across both DGE queues.
"""

import numpy as np

import concourse.bass as bass
import concourse.mybir as mybir
import concourse.tile as tile
from concourse import bacc
from concourse import bass_utils

P = 128
B = 4
C = 256
N = 4096          # 64*64 pixels
NJ = 2048         # j columns per core
NJT = NJ // P     # 16 j tiles
SCALE = 1.0 / np.sqrt(C).item()   # 1/16

F32 = mybir.dt.float32
BF16 = mybir.dt.bfloat16
AF = mybir.ActivationFunctionType

# x column blocks: (lo, width); packed host-side as [128, 2, w] per block
XBLK = [(0, 512), (512, 512), (1024, 1024), (2048, 1024), (3072, 1024)]
XOFF = [0]
for _lo, _w in XBLK:
    XOFF.append(XOFF[-1] + 2 * _w)      # free-dim offset into xp rows


def _build_module():
    nc = bacc.Bacc("TRN2", target_bir_lowering=False, debug=False, num_devices=8)

    xp_t = nc.dram_tensor("xp", [P, 2 * N], BF16, kind="ExternalInput")
    # wk rows first (slots 0,1), then wq (2,3), w2 (4,5), pad (6,7)
    wp_t = nc.dram_tensor("wp", [P, 8 * C], BF16, kind="ExternalInput")
    bp_t = nc.dram_tensor("bp", [P, 4 + C], F32, kind="ExternalInput")
    y_t = nc.dram_tensor("y", [C, N], BF16, kind="ExternalOutput")

    with tile.TileContext(nc) as tc:
        _emit(nc, tc, xp_t, wp_t, bp_t, y_t)
    nc.compile()
    return nc


def _emit(nc, tc, xp_t, wp_t, bp_t, y_t):
    from contextlib import ExitStack

    with ExitStack() as top:
        const = top.enter_context(tc.tile_pool(name="const", bufs=1))
        big = top.enter_context(tc.tile_pool(name="big", bufs=1))

        # ---- constants: host-packed; wk first so k-proj starts earliest --
        w_all = const.tile([P, 8, C], BF16, tag="w_all", name="w_all")
        nc.sync.dma_start(
            w_all[:, 0:2, :].rearrange("p w f -> p (w f)"),
            bass.AP(tensor=wp_t, offset=0, ap=[[8 * C, P], [1, 2 * C]]),
        )

        b_pack = const.tile([P, 4 + C], F32, tag="b_pack", name="b_pack")
        # (b_pack's DMA is issued below, after xb1 — biases are needed
        # ~2us later than the k-projection's x operand)
        b_all = b_pack[:, 0:4]       # cols: 0,1 = bq halves; 2,3 = bk halves
        bv_sb = b_pack[:, 4:4 + C]   # w2-folded v bias, broadcast to partitions

        # warmup tile in the const pool: its SBUF range is never recycled,
        # so the x-block DMAs can't pick up a WAR dependency on the warmups
        wsb = const.tile([P, 512], BF16, tag="wsb", name="wsb")

        # slot indices into w_all: wk=0, wq=1, w2=2
        def wslice(w, ci, ch):   # lhsT [128 ci, 128 co] for co half ch
            slot = {0: 2, 1: 0, 2: 4}[w]   # emitted order: wk first
            return w_all[:, slot + ci, ch * P:(ch + 1) * P]

        # ---- persistent activations -----------------------------------
        q_bf = [big.tile([P, N], BF16, tag=f"q{ch}", name=f"q{ch}") for ch in range(2)]
        k_bf = [big.tile([P, NJ], BF16, tag=f"k{ch}", name=f"k{ch}") for ch in range(2)]
        v_all = big.tile([P, NJT, C], BF16, tag="v_all", name="v_all")
        attn = [big.tile([P, N], BF16, tag=f"a{jt}", name=f"a{jt}") for jt in range(NJT)]
        # cols 0:64 = per-(jt,iq) exp sums, 64:80 = D, 80:96 = 1/D
        d_all = big.tile([P, 96], F32, tag="d_all", name="d_all")
        dsum_all = d_all[:, 64:96]

        # ---- warmups: lift the PE HAM clock-gate while x streams in ----
        # The HAM needs ~3.4us of *sustained* PE activity to unthrottle;
        # enough dummy matmuls to bridge until the first x block lands
        # (~12.5us) keep the busy window unbroken, so the projections run
        # at 2.4GHz from the start instead of paying ~5us of 1.2GHz ramp.
        with tc.tile_pool(name="warm_ps", bufs=1, space="PSUM") as wpp:
            wps = wpp.tile([P, 512], F32, tag="wps", name="wps")
            nc.vector.memset(wsb[:], 0.0)
            for _ in range(11):
                nc.tensor.matmul(wps[:], wsb[:, 0:P], wsb[:],
                                 start=True, stop=True)
            # dummy exp reads SBUF (not the warm PSUM): pulls the ~2.7us
            # ACT table load early without adding a PSUM dependency
            nc.scalar.activation(wsb[:, 508:509], wsb[:, 0:1], AF.Exp, scale=0.0)

        # rest of the weights after the first x block is enqueued below
        def bias_store(out_ap, ps, bias_ap, on_act):
            if on_act:
                nc.scalar.activation(out_ap, ps, AF.Identity, bias=bias_ap)
            else:
                nc.vector.tensor_scalar_add(out_ap, ps, bias_ap)

        psp = top.enter_context(tc.tile_pool(name="ps_s", bufs=2, space="PSUM"))

        def s_tile(jt, iq):
            # one [128,1024] score tile + exp(+accum) into the attn store
            ps = psp.tile([P, 1024], F32, tag="s", name="s_ps")
            for ch in range(2):
                lhs = k_bf[ch][:, jt * P:(jt + 1) * P]
                for t in range(2):
                    nc.tensor.matmul(
                        ps[:, t * 512:(t + 1) * 512], lhs,
                        q_bf[ch][:, iq * 1024 + t * 512: iq * 1024 + (t + 1) * 512],
                        start=(ch == 0), stop=(ch == 1),
                    )
            nc.scalar.activation(
                attn[jt][:, iq * 1024:(iq + 1) * 1024], ps[:],
                AF.Exp, scale=float(SCALE),
                accum_out=d_all[:, jt * 4 + iq: jt * 4 + iq + 1],
            )

        with tc.tile_pool(name="xload", bufs=1) as xp:
            xb = [xp.tile([P, 2, w], BF16, tag=f"xb{b}", name=f"xb{b}")
                  for b, (lo, w) in enumerate(XBLK)]
            # block0 right behind wk on the SP queue; block1 behind bp on ACT
            nc.sync.dma_start(
                xb[0][:], bass.AP(tensor=xp_t, offset=XOFF[0],
                                  ap=[[2 * N, P], [1, 2 * XBLK[0][1]]]))
            nc.scalar.dma_start(
                xb[1][:], bass.AP(tensor=xp_t, offset=XOFF[1],
                                  ap=[[2 * N, P], [1, 2 * XBLK[1][1]]]))
            nc.scalar.dma_start(
                b_pack[:],
                bass.AP(tensor=bp_t, offset=0, ap=[[4 + C, P], [1, 4 + C]]))
            # wq right after block0 on SP (q-proj needs it ~1us later),
            # then w2; both small so xb2 isn't pushed far back
            nc.sync.dma_start(
                w_all[:, 2:4, :].rearrange("p w f -> p (w f)"),
                bass.AP(tensor=wp_t, offset=2 * C, ap=[[8 * C, P], [1, 2 * C]]),
            )
            nc.sync.dma_start(
                w_all[:, 4:6, :].rearrange("p w f -> p (w f)"),
                bass.AP(tensor=wp_t, offset=4 * C, ap=[[8 * C, P], [1, 2 * C]]),
            )
            # xb2 carries k/v columns 1024-2047 (needed by blk1 of the
            # projections, ~12.5us) — put it on the lighter ACT queue ahead
            # of xb3; xb4 (q-only columns, needed last) rides on SP
            for b, eng in ((2, nc.scalar), (3, nc.scalar), (4, nc.sync)):
                eng.dma_start(
                    xb[b][:], bass.AP(tensor=xp_t, offset=XOFF[b],
                                      ap=[[2 * N, P], [1, 2 * XBLK[b][1]]]))

            def xsl(ci, lo, size):
                for b, (blo, w) in enumerate(XBLK):
                    if blo <= lo and lo + size <= blo + w:
                        return xb[b][:, ci, lo - blo:lo - blo + size]
                raise AssertionError((lo, size))

            # ---- phase 1: k, q, vp projections, emitted block-wise -------
            # Early score tiles are interleaved one-per-group so the PE
            # queue never head-of-line blocks on ACT's exp backlog; tile
            # (jt, iq) is only emitted once blk iq+? has its q chunks, i.e.
            # iq 0 during blk1, iq 1 during blk2, iq 2 during blk3.
            early_s = [(jt0, iq0) for iq0 in range(3) for jt0 in range(4)]
            es_i = 0

            def emit_early_s(n, limit):
                nonlocal es_i
                while n > 0 and es_i < limit:
                    s_tile(*early_s[es_i])
                    es_i += 1
                    n -= 1

            with tc.tile_pool(name="ps_qkv", bufs=4, space="PSUM") as pq:
                for blk in range(4):
                    if blk < 2:
                        for ch in range(2):
                            pss = [pq.tile([P, 512], F32, tag="ps", name="ps") for _ in range(2)]
                            for ci in range(2):
                                lhs = wslice(1, ci, ch)
                                for t2 in range(2):
                                    t = blk * 2 + t2
                                    nc.tensor.matmul(
                                        pss[t2][:], lhs,
                                        xsl(ci, t * 512, 512),
                                        start=(ci == 0), stop=(ci == 1),
                                    )
                            for t2 in range(2):
                                t = blk * 2 + t2
                                bias_store(k_bf[ch][:, t * 512:(t + 1) * 512], pss[t2][:],
                                           b_all[:, 2 + ch:3 + ch], on_act=(ch == 0))
                    for ch in range(2):
                        pss = [pq.tile([P, 512], F32, tag="ps", name="ps") for _ in range(2)]
                        for ci in range(2):
                            lhs = wslice(0, ci, ch)
                            for t2 in range(2):
                                ic = blk * 2 + t2
                                nc.tensor.matmul(
                                    pss[t2][:], lhs,
                                    xsl(ci, ic * 512, 512),
                                    start=(ci == 0), stop=(ci == 1),
                                )
                        for t2 in range(2):
                            ic = blk * 2 + t2
                            bias_store(q_bf[ch][:, ic * 512:(ic + 1) * 512], pss[t2][:],
                                       b_all[:, ch:ch + 1], on_act=(ch == 0))
                        if blk >= 1:
                            emit_early_s(1, 4 * blk)
                    if blk < 2:
                        for jtg in range(2):
                            pss = [pq.tile([P, C], F32, tag="ps", name="ps") for _ in range(4)]
                            for ci in range(2):
                                for t in range(4):
                                    jt = blk * 8 + jtg * 4 + t
                                    nc.tensor.matmul(
                                        pss[t][:],
                                        xsl(ci, jt * P, P),
                                        w_all[:, 4 + ci, :],
                                        start=(ci == 0), stop=(ci == 1),
                                    )
                            for t in range(4):
                                nc.vector.tensor_add(
                                    v_all[:, blk * 8 + jtg * 4 + t, :], pss[t][:], bv_sb[:]
                                )
                            if blk == 1:
                                emit_early_s(1, 4 * blk)
                    if blk >= 2:
                        emit_early_s(2, 4 * blk)
                emit_early_s(len(early_s), len(early_s))  # remainder

        # ---- phase 2+3 fused: scores/exp interleaved with y accum ------
        with tc.tile_pool(name="yaccp", bufs=1) as yp, \
             tc.tile_pool(name="ps_o", bufs=2, space="PSUM") as po, \
             tc.tile_pool(name="ysb", bufs=2) as ysb_pool:
            y_acc = yp.tile([P, 8, 1024], F32, tag="y_acc", name="y_acc")

            def out_chain(g, idx):
                # one accumulation chain: jts 4g..4g+3 into (iq, ch) slice
                iq, ch = divmod(idx, 2)
                ops = po.tile([P, 1024], F32, tag="og", name="og")
                for j2 in range(4 * g, 4 * g + 4):
                    lhs = v_all[:, j2, ch * P:(ch + 1) * P]
                    for t in range(2):
                        nc.tensor.matmul(
                            ops[:, t * 512:(t + 1) * 512], lhs,
                            attn[j2][:, iq * 1024 + t * 512: iq * 1024 + (t + 1) * 512],
                            start=(j2 == 4 * g), stop=(j2 == 4 * g + 3),
                        )
                if g == 0:
                    nc.vector.tensor_copy(y_acc[:, idx, :], ops[:])
                elif g < 3:
                    nc.vector.tensor_add(y_acc[:, idx, :], ops[:], y_acc[:, idx, :])
                else:
                    # split the final add + store into 512-halves so the
                    # last store isn't serialized behind one 1024-wide add
                    y_sb = ysb_pool.tile([P, 1024], BF16, tag="ysb", name="ysb")
                    eng = nc.sync if idx % 2 == 0 else nc.scalar
                    for h in range(2):
                        hs = slice(h * 512, (h + 1) * 512)
                        nc.vector.tensor_add(
                            y_sb[:, hs], ops[:, hs], y_acc[:, idx, hs])
                        eng.dma_start(
                            y_t.ap()[ch * P:(ch + 1) * P,
                                     iq * 1024 + h * 512: iq * 1024 + (h + 1) * 512],
                            y_sb[:, hs],
                        )

            for jt in range(NJT):
                for iq in range(4):
                    if jt < 4 and iq < 3:
                        continue  # pre-emitted during the qkv phase
                    s_tile(jt, iq)
                # per-jt denominator (sum the 4 chunk sums) + vp scaling
                nc.vector.reduce_sum(
                    dsum_all[:, jt:jt + 1], d_all[:, jt * 4:jt * 4 + 4],
                    axis=mybir.AxisListType.X,
                )
                nc.vector.reciprocal(
                    dsum_all[:, 16 + jt:17 + jt], dsum_all[:, jt:jt + 1]
                )
                nc.vector.tensor_scalar_mul(
                    v_all[:, jt, :], v_all[:, jt, :],
                    dsum_all[:, 16 + jt:17 + jt],
                )
                if jt >= 4:
                    g = jt // 4 - 1
                    off = (jt % 4) * 2
                    out_chain(g, off)
                    out_chain(g, off + 1)
            for idx in range(8):
                out_chain(3, idx)

_nc_cache = None
LAST_EXEC_TIME_NS = None


def _get_nc():
    global _nc_cache
    if _nc_cache is None:
        _nc_cache = _build_module()
    return _nc_cache


def _pack_x(xb):
    # xb [C, N] bf16 -> [128, sum(2*w)] with per-block [p, ci, w] layout
    cols = []
    for lo, w in XBLK:
        blkv = xb[:, lo:lo + w].reshape(2, P, w).transpose(1, 0, 2)
        cols.append(np.ascontiguousarray(blkv).reshape(P, 2 * w))
    return np.ascontiguousarray(np.concatenate(cols, axis=1))


def kernel(x, wq, bq, wk, bk, wv, bv, wp, bp):
    global LAST_EXEC_TIME_NS
    nc = _get_nc()

    import ml_dtypes
    bf = ml_dtypes.bfloat16
    x = np.asarray(x, dtype=np.float32).reshape(B, C, N).astype(bf)
    wq32 = np.asarray(wq, dtype=np.float32)
    wk32 = np.asarray(wk, dtype=np.float32)
    wv32 = np.asarray(wv, dtype=np.float32)
    wp32 = np.asarray(wp, dtype=np.float32)
    w2 = wp32 @ wv32                      # fold the output projection into v

    # w_pack slot order: wk(0,1), wq(2,3), w2(4,5), pad(6,7)
    wT = np.stack([wk32.T, wq32.T, w2.T]).astype(bf)    # [3, 256 ci, 256 co]
    w_pack = np.zeros((P, 8, C), dtype=bf)
    for w in range(3):
        w_pack[:, 2 * w:2 * w + 2, :] = wT[w].reshape(2, P, C).transpose(1, 0, 2)
    w_pack = np.ascontiguousarray(w_pack.reshape(P, 8 * C))

    bq32 = np.asarray(bq, dtype=np.float32)
    bk32 = np.asarray(bk, dtype=np.float32)
    b_pack = np.zeros((P, 4 + C), dtype=np.float32)
    b_pack[:, 0] = bq32[:P]
    b_pack[:, 1] = bq32[P:]
    b_pack[:, 2] = bk32[:P]
    b_pack[:, 3] = bk32[P:]
    b_pack[:, 4:] = (wp32 @ np.asarray(bv, dtype=np.float32))[None, :]
    b_pack = np.ascontiguousarray(b_pack)

    bp1 = np.asarray(bp, dtype=np.float32).reshape(C)

    in_maps = []
    for core in range(8):
        b, h = divmod(core, 2)
        xb = x[b] if h == 0 else np.ascontiguousarray(np.roll(x[b], -NJ, axis=1))
        in_maps.append({"xp": _pack_x(xb), "wp": w_pack, "bp": b_pack})

    res = bass_utils.run_bass_kernel_spmd(nc, in_maps, core_ids=list(range(8)))
    if res.exec_time_ns is not None:
        LAST_EXEC_TIME_NS = res.exec_time_ns

    y = np.zeros((B, C, N), np.float32)
    for b in range(B):
        ya = res.results[2 * b]["y"].astype(np.float32)
        yb = res.results[2 * b + 1]["y"].astype(np.float32)
        y[b] = ya + np.roll(yb, NJ, axis=1)
    y += bp1.reshape(1, C, 1)
    return y.reshape(B, C, 64, 64)



# revision 12
# speedup vs baseline: 1.0200x; 1.0200x over previous
"""Trainium2 Bass kernel for the AttnBlock-style attention module.

Reference computation (softmax over axis=1, the *i* axis):
    q = wq @ x + bq ; k = wk @ x + bk ; v = wv @ x + bv      (per-pixel 1x1 conv)
    s[b,i,j] = (q[b,:,i] . k[b,:,j]) * C**-0.5
    attn = softmax_i(s)                                      (normalize over i!)
    out[b,c,i] = sum_j attn[b,i,j] v[b,c,j]
    y = wp @ out + bp

Sharding: 8 cores = 4 batches x 2 j-halves.  Softmax normalizes over i, so
each core owns a full-i x half-j block and its denominators D_j are local.

Numerics / performance design (v2):
  - x / weights / q / k in fp16 (same PE rate as bf16, 8x finer mantissa).
  - wp is folded into wv host-side (w2 = wp@wv, scaled by ALPHA) so the
    attention output needs no second projection; host divides by ALPHA.
  - exp(s*scale - C_SHIFT) written directly as fp8e4 (max ~162 < 240), the
    shift cancels in the j-local normalization.  ACT runs *only* exp.
  - attn @ v runs in fp8 DoubleRow mode (K=256 per instruction): v is scaled
    by ALPHA/D_j and quantized to fp8; OUT_MODE="dr2" adds an fp8 residual
    pass for v (hi/lo split) at twice the out-matmul cost.
  - out accumulates over j in PSUM in two j-groups (12+4 jt), staged through
    SBUF once; chains are emitted pairwise so consecutive PE instructions
    never hit the same PSUM region (RMW hazard).
"""

import numpy as np

import concourse.bass as bass
import concourse.mybir as mybir
import concourse.tile as tile
from concourse import bacc
from concourse import bass_utils

P = 128
B = 4
C = 256
N = 4096          # 64*64 pixels
NJ = 2048         # j columns per core
NJT = NJ // P     # 16 j tiles
SCALE = 1.0 / np.sqrt(C).item()   # 1/16
C_SHIFT = 3.0                     # exp(s - C_SHIFT): max ~e^5.1=162 < 240
ALPHA = 4096.0                    # v pre-scale, divided out on host

OUT_MODE = "dr1"                  # "dr1" | "dr2" (adds v-residual pass)

F32 = mybir.dt.float32
FP16 = mybir.dt.float16
FP8 = mybir.dt.float8e4
AF = mybir.ActivationFunctionType
DR = mybir.MatmulPerfMode.DoubleRow

# x column blocks: (lo, width); packed host-side as [128, 2, w] per block
XBLK = [(0, 512), (512, 512), (1024, 1024), (2048, 1024), (3072, 1024)]
XOFF = [0]
for _lo, _w in XBLK:
    XOFF.append(XOFF[-1] + 2 * _w)      # free-dim offset into xp rows


def _build_module():
    nc = bacc.Bacc("TRN2", target_bir_lowering=False, debug=False, num_devices=8)

    xp_t = nc.dram_tensor("xp", [P, 2 * N], FP16, kind="ExternalInput")
    # wk rows first (slots 0,1), then wq (2,3), w2*ALPHA (4,5), pad (6,7)
    wp_t = nc.dram_tensor("wp", [P, 8 * C], FP16, kind="ExternalInput")
    bp_t = nc.dram_tensor("bp", [P, 4 + C], F32, kind="ExternalInput")
    y_t = nc.dram_tensor("y", [C, N], FP16, kind="ExternalOutput")

    with tile.TileContext(nc) as tc:
        _emit(nc, tc, xp_t, wp_t, bp_t, y_t)
    nc.compile()
    return nc


def _emit(nc, tc, xp_t, wp_t, bp_t, y_t):
    from contextlib import ExitStack

    with ExitStack() as top:
        const = top.enter_context(tc.tile_pool(name="const", bufs=1))
        big = top.enter_context(tc.tile_pool(name="big", bufs=1))

        # ---- constants: host-packed; wk first so k-proj starts earliest --
        w_all = const.tile([P, 8, C], FP16, tag="w_all", name="w_all")
        nc.sync.dma_start(
            w_all[:, 0:2, :].rearrange("p w f -> p (w f)"),
            bass.AP(tensor=wp_t, offset=0, ap=[[8 * C, P], [1, 2 * C]]),
        )

        b_pack = const.tile([P, 4 + C], F32, tag="b_pack", name="b_pack")
        b_all = b_pack[:, 0:4]       # cols: 0,1 = bq halves; 2,3 = bk halves
        bv_sb = b_pack[:, 4:4 + C]   # ALPHA*(wp@bv), broadcast to partitions

        # warmup tile in the const pool (never recycled -> no WAR on x DMAs)
        wsb = const.tile([P, 512], FP16, tag="wsb", name="wsb")

        # per-partition exp bias (-C_SHIFT); floats other than 0/1 have no
        # pre-registered const AP
        cbias = const.tile([P, 1], F32, tag="cbias", name="cbias")
        nc.vector.memset(cbias[:], -C_SHIFT)

        # slot indices into w_all: wk=0, wq=1, w2=2
        def wslice(w, ci, ch):   # lhsT [128 ci, 128 co] for co half ch
            slot = {0: 2, 1: 0, 2: 4}[w]   # emitted order: wk first
            return w_all[:, slot + ci, ch * P:(ch + 1) * P]

        # ---- persistent activations -----------------------------------
        q_h = [big.tile([P, N], FP16, tag=f"q{ch}", name=f"q{ch}") for ch in range(2)]
        k_h = [big.tile([P, NJ], FP16, tag=f"k{ch}", name=f"k{ch}") for ch in range(2)]
        v_h = big.tile([P, NJT, C], FP16, tag="v_h", name="v_h")
        # attn fp8, pair-contiguous for DoubleRow: [pair, ichunk512, parity, 512]
        attn8 = big.tile([P, 8, 8, 2, 512], FP8, tag="attn8", name="attn8")
        # v fp8 (and residual for dr2): [pair, ch, parity, 128]
        v8 = big.tile([P, 8, 2, 2, P], FP8, tag="v8", name="v8")
        if OUT_MODE == "dr2":
            v8l = big.tile([P, 8, 2, 2, P], FP8, tag="v8l", name="v8l")
            vtmp = big.tile([P, C], F32, tag="vtmp", name="vtmp")
        # cols 0:64 = per-(jt,iq) exp sums, 64:80 = D, 80:96 = 1/D
        d_all = big.tile([P, 96], F32, tag="d_all", name="d_all")
        dsum_all = d_all[:, 64:96]
        y_acc = big.tile([P, 16, 512], F32, tag="y_acc", name="y_acc")

        # ---- warmups: lift the PE HAM clock-gate while x streams in ----
        with tc.tile_pool(name="warm_ps", bufs=1, space="PSUM") as wpp:
            wps = wpp.tile([P, 512], F32, tag="wps", name="wps")
            nc.vector.memset(wsb[:], 0.0)
            for i in range(11):
                nc.tensor.matmul(wps[:, 0:256] if i % 2 == 0 else wps[:, 256:512],
                                 wsb[:, 0:P], wsb[:, 0:256],
                                 start=True, stop=True)
            # dummy exp reads SBUF: pulls the ACT table load early
            nc.scalar.activation(wsb[:, 508:509], wsb[:, 0:1], AF.Exp, scale=0.0)

        psp = top.enter_context(tc.tile_pool(name="ps_s", bufs=3, space="PSUM"))

        def s_tile(jt, iq):
            # one [128,1024] score tile + exp into the fp8 attn store
            pr, par = jt >> 1, jt & 1
            ps = psp.tile([P, 1024], F32, tag="s", name="s_ps")
            for ch in range(2):
                lhs = k_h[ch][:, jt * P:(jt + 1) * P]
                for t in range(2):
                    nc.tensor.matmul(
                        ps[:, t * 512:(t + 1) * 512], lhs,
                        q_h[ch][:, iq * 1024 + t * 512: iq * 1024 + (t + 1) * 512],
                        start=(ch == 0), stop=(ch == 1),
                    )
            nc.scalar.activation(
                attn8[:, pr, 2 * iq:2 * iq + 2, par, :],
                ps[:].rearrange("p (a f) -> p a f", a=2),
                AF.Exp, scale=float(SCALE), bias=cbias[:],
                accum_out=d_all[:, jt * 4 + iq: jt * 4 + iq + 1],
            )

        def finish_jt(jt):
            # denominator, reciprocal, v scaling -> fp8 (on DVE)
            pr, par = jt >> 1, jt & 1
            nc.vector.reduce_sum(
                dsum_all[:, jt:jt + 1], d_all[:, jt * 4:jt * 4 + 4],
                axis=mybir.AxisListType.X,
            )
            nc.vector.reciprocal(
                dsum_all[:, 16 + jt:17 + jt], dsum_all[:, jt:jt + 1]
            )
            vin = v_h[:, jt, :].rearrange("p (c f) -> p c f", c=2)
            if OUT_MODE == "dr2":
                nc.vector.tensor_scalar_mul(
                    vtmp[:], v_h[:, jt, :], dsum_all[:, 16 + jt:17 + jt])
                vt2 = vtmp[:].rearrange("p (c f) -> p c f", c=2)
                nc.vector.tensor_copy(v8[:, pr, :, par, :], vt2)
                nc.vector.tensor_tensor(
                    v8l[:, pr, :, par, :], vt2, v8[:, pr, :, par, :],
                    mybir.AluOpType.subtract,
                )
            else:
                nc.vector.tensor_scalar_mul(
                    v8[:, pr, :, par, :], vin,
                    dsum_all[:, 16 + jt:17 + jt])

        with tc.tile_pool(name="xload", bufs=1) as xp:
            xb = [xp.tile([P, 2, w], FP16, tag=f"xb{b}", name=f"xb{b}")
                  for b, (lo, w) in enumerate(XBLK)]
            # block0 right behind wk on the SP queue; block1 + biases on DVE q
            nc.sync.dma_start(
                xb[0][:], bass.AP(tensor=xp_t, offset=XOFF[0],
                                  ap=[[2 * N, P], [1, 2 * XBLK[0][1]]]))
            nc.scalar.dma_start(
                b_pack[:],
                bass.AP(tensor=bp_t, offset=0, ap=[[4 + C, P], [1, 4 + C]]))
            nc.scalar.dma_start(
                xb[1][:], bass.AP(tensor=xp_t, offset=XOFF[1],
                                  ap=[[2 * N, P], [1, 2 * XBLK[1][1]]]))
            nc.sync.dma_start(
                w_all[:, 2:4, :].rearrange("p w f -> p (w f)"),
                bass.AP(tensor=wp_t, offset=2 * C, ap=[[8 * C, P], [1, 2 * C]]),
            )
            nc.sync.dma_start(
                w_all[:, 4:6, :].rearrange("p w f -> p (w f)"),
                bass.AP(tensor=wp_t, offset=4 * C, ap=[[8 * C, P], [1, 2 * C]]),
            )
            for b, eng in ((2, nc.scalar), (3, nc.scalar), (4, nc.sync)):
                eng.dma_start(
                    xb[b][:], bass.AP(tensor=xp_t, offset=XOFF[b],
                                      ap=[[2 * N, P], [1, 2 * XBLK[b][1]]]))

            def xsl(ci, lo, size):
                for b, (blo, w) in enumerate(XBLK):
                    if blo <= lo and lo + size <= blo + w:
                        return xb[b][:, ci, lo - blo:lo - blo + size]
                raise AssertionError((lo, size))

            # ---- phase 1: k, q, v projections (fp16), early s_tiles ----
            early_s = [(jt0, iq0) for iq0 in range(3) for jt0 in range(4)]
            es_i = 0

            def emit_early_s(n, limit):
                nonlocal es_i
                while n > 0 and es_i < limit:
                    s_tile(*early_s[es_i])
                    es_i += 1
                    n -= 1

            with tc.tile_pool(name="ps_qkv", bufs=2, space="PSUM") as pq:
                for blk in range(4):
                    if blk < 2:
                        for ch in range(2):
                            pss = [pq.tile([P, 512], F32, tag="ps", name="ps") for _ in range(2)]
                            for ci in range(2):
                                lhs = wslice(1, ci, ch)
                                for t2 in range(2):
                                    t = blk * 2 + t2
                                    nc.tensor.matmul(
                                        pss[t2][:], lhs,
                                        xsl(ci, t * 512, 512),
                                        start=(ci == 0), stop=(ci == 1),
                                    )
                            for t2 in range(2):
                                t = blk * 2 + t2
                                nc.vector.tensor_scalar_add(
                                    k_h[ch][:, t * 512:(t + 1) * 512], pss[t2][:],
                                    b_all[:, 2 + ch:3 + ch])
                    for ch in range(2):
                        pss = [pq.tile([P, 512], F32, tag="ps", name="ps") for _ in range(2)]
                        for ci in range(2):
                            lhs = wslice(0, ci, ch)
                            for t2 in range(2):
                                ic = blk * 2 + t2
                                nc.tensor.matmul(
                                    pss[t2][:], lhs,
                                    xsl(ci, ic * 512, 512),
                                    start=(ci == 0), stop=(ci == 1),
                                )
                        for t2 in range(2):
                            ic = blk * 2 + t2
                            nc.vector.tensor_scalar_add(
                                q_h[ch][:, ic * 512:(ic + 1) * 512], pss[t2][:],
                                b_all[:, ch:ch + 1])
                        if blk >= 1:
                            emit_early_s(1, 4 * blk)
                    if blk < 2:
                        for jtg in range(2):
                            pss = [pq.tile([P, C], F32, tag="ps", name="ps") for _ in range(4)]
                            for ci in range(2):
                                for t in range(4):
                                    jt = blk * 8 + jtg * 4 + t
                                    nc.tensor.matmul(
                                        pss[t][:],
                                        xsl(ci, jt * P, P),
                                        w_all[:, 4 + ci, :],
                                        start=(ci == 0), stop=(ci == 1),
                                    )
                            for t in range(4):
                                nc.vector.tensor_add(
                                    v_h[:, blk * 8 + jtg * 4 + t, :], pss[t][:], bv_sb[:]
                                )
                            if blk == 1:
                                emit_early_s(1, 4 * blk)
                    if blk >= 2:
                        emit_early_s(2, 4 * blk)
                emit_early_s(len(early_s), len(early_s))  # remainder

        # ---- phase 2+3: scores/exp interleaved with fp8 DR out accum ---
        # groups: g0 = pairs 0..5 (jt 0..11), g1 = pairs 6..7 (jt 12..15)
        with tc.tile_pool(name="ps_o", bufs=2, space="PSUM") as po, \
             tc.tile_pool(name="ysb", bufs=2) as ysb_pool:

            def out_chain_pair(g, z):
                # two interleaved chains (z, ch=0) and (z, ch=1); z = i512 chunk
                prs = range(0, 6) if g == 0 else range(6, 8)
                ops = [po.tile([P, 512], F32, tag="og", name="og") for _ in range(2)]
                npr = len(prs)
                for si, pr in enumerate(prs):
                    for ch in range(2):
                        nc.tensor.matmul(
                            ops[ch][:], v8[:, pr, ch, :, :], attn8[:, pr, z, :, :],
                            start=(si == 0), stop=(si == npr - 1 and OUT_MODE != "dr2"),
                            perf_mode=DR,
                        )
                if OUT_MODE == "dr2":
                    for si, pr in enumerate(prs):
                        for ch in range(2):
                            nc.tensor.matmul(
                                ops[ch][:], v8l[:, pr, ch, :, :], attn8[:, pr, z, :, :],
                                start=False, stop=(si == npr - 1),
                                perf_mode=DR,
                            )
                for ch in range(2):
                    idx = z * 2 + ch
                    if g == 0:
                        nc.vector.tensor_copy(y_acc[:, idx, :], ops[ch][:])
                    else:
                        y_sb = ysb_pool.tile([P, 512], FP16, tag="ysb", name="ysb")
                        nc.vector.tensor_add(y_sb[:], ops[ch][:], y_acc[:, idx, :])
                        eng = nc.sync if ch == 0 else nc.scalar
                        eng.dma_start(
                            y_t.ap()[ch * P:(ch + 1) * P, z * 512:(z + 1) * 512],
                            y_sb[:],
                        )

            for jt in range(NJT):
                for iq in range(4):
                    if jt < 4 and iq < 3:
                        continue  # pre-emitted during the qkv phase
                    s_tile(jt, iq)
                finish_jt(jt)
                if jt >= 12:
                    z0 = (jt - 12) * 2
                    out_chain_pair(0, z0)
                    out_chain_pair(0, z0 + 1)
            for z in range(8):
                out_chain_pair(1, z)


_nc_cache = None
LAST_EXEC_TIME_NS = None
LAST_RESULT = None


def _get_nc():
    global _nc_cache
    if _nc_cache is None:
        _nc_cache = _build_module()
    return _nc_cache


def _pack_x(xb):
    # xb [C, N] fp16 -> [128, sum(2*w)] with per-block [p, ci, w] layout
    cols = []
    for lo, w in XBLK:
        blkv = xb[:, lo:lo + w].reshape(2, P, w).transpose(1, 0, 2)
        cols.append(np.ascontiguousarray(blkv).reshape(P, 2 * w))
    return np.ascontiguousarray(np.concatenate(cols, axis=1))


def kernel(x, wq, bq, wk, bk, wv, bv, wp, bp):
    global LAST_EXEC_TIME_NS, LAST_RESULT
    nc = _get_nc()

    f16 = np.float16
    x = np.asarray(x, dtype=np.float32).reshape(B, C, N).astype(f16)
    wq32 = np.asarray(wq, dtype=np.float32)
    wk32 = np.asarray(wk, dtype=np.float32)
    wv32 = np.asarray(wv, dtype=np.float32)
    wp32 = np.asarray(wp, dtype=np.float32)
    w2 = (wp32 @ wv32) * ALPHA            # fold output projection + ALPHA into v

    # w_pack slot order: wk(0,1), wq(2,3), w2(4,5), pad(6,7)
    wT = np.stack([wk32.T, wq32.T, w2.T]).astype(f16)    # [3, 256 ci, 256 co]
    w_pack = np.zeros((P, 8, C), dtype=f16)
    for w in range(3):
        w_pack[:, 2 * w:2 * w + 2, :] = wT[w].reshape(2, P, C).transpose(1, 0, 2)
    w_pack = np.ascontiguousarray(w_pack.reshape(P, 8 * C))

    bq32 = np.asarray(bq, dtype=np.float32)
    bk32 = np.asarray(bk, dtype=np.float32)
    b_pack = np.zeros((P, 4 + C), dtype=np.float32)
    b_pack[:, 0] = bq32[:P]
    b_pack[:, 1] = bq32[P:]
    b_pack[:, 2] = bk32[:P]
    b_pack[:, 3] = bk32[P:]
    b_pack[:, 4:] = (ALPHA * (wp32 @ np.asarray(bv, dtype=np.float32)))[None, :]
    b_pack = np.ascontiguousarray(b_pack)

    bp1 = np.asarray(bp, dtype=np.float32).reshape(C)

    in_maps = []
    for core in range(8):
        b, h = divmod(core, 2)
        xb = x[b] if h == 0 else np.ascontiguousarray(np.roll(x[b], -NJ, axis=1))
        in_maps.append({"xp": _pack_x(xb), "wp": w_pack, "bp": b_pack})

    res = bass_utils.run_bass_kernel_spmd(nc, in_maps, core_ids=list(range(8)))
    if res.exec_time_ns is not None:
        LAST_EXEC_TIME_NS = res.exec_time_ns
    LAST_RESULT = res

    y = np.zeros((B, C, N), np.float32)
    for b in range(B):
        ya = res.results[2 * b]["y"].astype(np.float32)
        yb = res.results[2 * b + 1]["y"].astype(np.float32)
        y[b] = ya + np.roll(yb, NJ, axis=1)
    y = y * (1.0 / ALPHA) + bp1.reshape(1, C, 1)
    return y.reshape(B, C, 64, 64)


# revision 15
# speedup vs baseline: 1.0895x; 1.0681x over previous
"""Trainium2 Bass kernel for the AttnBlock-style attention module.

Reference computation (softmax over axis=1, the *i* axis):
    q = wq @ x + bq ; k = wk @ x + bk ; v = wv @ x + bv      (per-pixel 1x1 conv)
    s[b,i,j] = (q[b,:,i] . k[b,:,j]) * C**-0.5
    attn = softmax_i(s)                                      (normalize over i!)
    out[b,c,i] = sum_j attn[b,i,j] v[b,c,j]
    y = wp @ out + bp

Sharding: 8 cores = 4 batches x 2 j-halves.  Softmax normalizes over i, so
each core owns a full-i x half-j block and its denominators D_j are local.

Numerics / performance design (v2):
  - x / weights / q / k in fp16 (same PE rate as bf16, 8x finer mantissa).
  - wp is folded into wv host-side (w2 = wp@wv, scaled by ALPHA) so the
    attention output needs no second projection; host divides by ALPHA.
  - exp(s*scale - C_SHIFT) written directly as fp8e4 (max ~162 < 240), the
    shift cancels in the j-local normalization.  ACT runs *only* exp.
  - attn @ v runs in fp8 DoubleRow mode (K=256 per instruction): v is scaled
    by ALPHA/D_j and quantized to fp8; OUT_MODE="dr2" adds an fp8 residual
    pass for v (hi/lo split) at twice the out-matmul cost.
  - out accumulates over j in PSUM in two j-groups (12+4 jt), staged through
    SBUF once; chains are emitted pairwise so consecutive PE instructions
    never hit the same PSUM region (RMW hazard).
"""

import numpy as np

import concourse.bass as bass
import concourse.mybir as mybir
import concourse.tile as tile
from concourse import bacc
from concourse import bass_utils

P = 128
B = 4
C = 256
N = 4096          # 64*64 pixels
NJ = 2048         # j columns per core
NJT = NJ // P     # 16 j tiles
SCALE = 1.0 / np.sqrt(C).item()   # 1/16
C_SHIFT = 3.0                     # exp(s - C_SHIFT): max ~e^5.1=162 < 240
ALPHA = 4096.0                    # v pre-scale, divided out on host

OUT_MODE = "dr1"                  # "dr1" | "dr2" (adds v-residual pass)

F32 = mybir.dt.float32
FP16 = mybir.dt.float16
FP8 = mybir.dt.float8e4
AF = mybir.ActivationFunctionType
DR = mybir.MatmulPerfMode.DoubleRow

# x column blocks: (lo, width); packed host-side as [128, 2, w] per block
XBLK = [(0, 512), (512, 512), (1024, 1024), (2048, 1024), (3072, 1024)]
XOFF = [0]
for _lo, _w in XBLK:
    XOFF.append(XOFF[-1] + 2 * _w)      # free-dim offset into xp rows


def _build_module():
    nc = bacc.Bacc("TRN2", target_bir_lowering=False, debug=False, num_devices=8)

    xp_t = nc.dram_tensor("xp", [P, 2 * N], FP16, kind="ExternalInput")
    # wk rows first (slots 0,1), then wq (2,3), w2*ALPHA (4,5), pad (6,7)
    wp_t = nc.dram_tensor("wp", [P, 8 * C], FP16, kind="ExternalInput")
    bp_t = nc.dram_tensor("bp", [P, 4 + C], F32, kind="ExternalInput")
    y_t = nc.dram_tensor("y", [C, N], FP16, kind="ExternalOutput")

    with tile.TileContext(nc) as tc:
        _emit(nc, tc, xp_t, wp_t, bp_t, y_t)
    nc.compile()
    return nc


def _emit(nc, tc, xp_t, wp_t, bp_t, y_t):
    from contextlib import ExitStack

    with ExitStack() as top:
        const = top.enter_context(tc.tile_pool(name="const", bufs=1))
        big = top.enter_context(tc.tile_pool(name="big", bufs=1))

        # ---- constants: host-packed; wk first so k-proj starts earliest --
        w_all = const.tile([P, 8, C], FP16, tag="w_all", name="w_all")
        nc.sync.dma_start(
            w_all[:, 0:2, :].rearrange("p w f -> p (w f)"),
            bass.AP(tensor=wp_t, offset=0, ap=[[8 * C, P], [1, 2 * C]]),
        )

        b_pack = const.tile([P, 4 + C], F32, tag="b_pack", name="b_pack")
        b_all = b_pack[:, 0:4]       # cols: 0,1 = bq halves; 2,3 = bk halves
        bv_sb = b_pack[:, 4:4 + C]   # ALPHA*(wp@bv), broadcast to partitions

        # warmup tile in the const pool (never recycled -> no WAR on x DMAs)
        wsb = const.tile([P, 512], FP16, tag="wsb", name="wsb")

        # per-partition exp bias (-C_SHIFT); floats other than 0/1 have no
        # pre-registered const AP
        cbias = const.tile([P, 1], F32, tag="cbias", name="cbias")
        nc.vector.memset(cbias[:], -C_SHIFT)

        # slot indices into w_all: wk=0, wq=1, w2=2
        def wslice(w, ci, ch):   # lhsT [128 ci, 128 co] for co half ch
            slot = {0: 2, 1: 0, 2: 4}[w]   # emitted order: wk first
            return w_all[:, slot + ci, ch * P:(ch + 1) * P]

        # ---- persistent activations -----------------------------------
        q_h = [big.tile([P, N], FP16, tag=f"q{ch}", name=f"q{ch}") for ch in range(2)]
        k_h = [big.tile([P, NJ], FP16, tag=f"k{ch}", name=f"k{ch}") for ch in range(2)]
        v_h = big.tile([P, NJT, C], FP16, tag="v_h", name="v_h")
        # attn fp8, pair-contiguous for DoubleRow: [pair, ichunk512, parity, 512]
        attn8 = big.tile([P, 8, 8, 2, 512], FP8, tag="attn8", name="attn8")
        # v fp8 (and residual for dr2): [pair, ch, parity, 128]
        v8 = big.tile([P, 8, 2, 2, P], FP8, tag="v8", name="v8")
        if OUT_MODE == "dr2":
            v8l = big.tile([P, 8, 2, 2, P], FP8, tag="v8l", name="v8l")
            vtmp = big.tile([P, C], F32, tag="vtmp", name="vtmp")
        # cols 0:64 = per-(jt,iq) exp sums, 64:80 = D, 80:96 = 1/D
        d_all = big.tile([P, 96], F32, tag="d_all", name="d_all")
        dsum_all = d_all[:, 64:96]
        y_acc = big.tile([P, 16, 512], F32, tag="y_acc", name="y_acc")
        # final fp16 output staging: [ch, z, 512]; 4 large DMAs at the end
        y_fin = big.tile([P, 2, 8, 512], FP16, tag="y_fin", name="y_fin")

        # ---- warmups: lift the PE HAM clock-gate while x streams in ----
        with tc.tile_pool(name="warm_ps", bufs=1, space="PSUM") as wpp:
            wps = wpp.tile([P, 512], F32, tag="wps", name="wps")
            nc.vector.memset(wsb[:], 0.0)
            for i in range(11):
                nc.tensor.matmul(wps[:, 0:256] if i % 2 == 0 else wps[:, 256:512],
                                 wsb[:, 0:P], wsb[:, 0:256],
                                 start=True, stop=True)
            # dummy exp reads SBUF: pulls the ACT table load early
            nc.scalar.activation(wsb[:, 508:509], wsb[:, 0:1], AF.Exp, scale=0.0)

        psp = top.enter_context(tc.tile_pool(name="ps_s", bufs=3, space="PSUM"))

        def s_tile(jt, iq):
            # one [128,1024] score tile + exp into the fp8 attn store
            pr, par = jt >> 1, jt & 1
            ps = psp.tile([P, 1024], F32, tag="s", name="s_ps")
            for ch in range(2):
                lhs = k_h[ch][:, jt * P:(jt + 1) * P]
                for t in range(2):
                    nc.tensor.matmul(
                        ps[:, t * 512:(t + 1) * 512], lhs,
                        q_h[ch][:, iq * 1024 + t * 512: iq * 1024 + (t + 1) * 512],
                        start=(ch == 0), stop=(ch == 1),
                    )
            nc.scalar.activation(
                attn8[:, pr, 2 * iq:2 * iq + 2, par, :],
                ps[:].rearrange("p (a f) -> p a f", a=2),
                AF.Exp, scale=float(SCALE), bias=cbias[:],
                accum_out=d_all[:, jt * 4 + iq: jt * 4 + iq + 1],
            )

        def finish_jt(jt):
            # denominator, reciprocal, v scaling -> fp8 (on DVE)
            pr, par = jt >> 1, jt & 1
            nc.vector.reduce_sum(
                dsum_all[:, jt:jt + 1], d_all[:, jt * 4:jt * 4 + 4],
                axis=mybir.AxisListType.X,
            )
            nc.vector.reciprocal(
                dsum_all[:, 16 + jt:17 + jt], dsum_all[:, jt:jt + 1]
            )
            vin = v_h[:, jt, :].rearrange("p (c f) -> p c f", c=2)
            if OUT_MODE == "dr2":
                nc.vector.tensor_scalar_mul(
                    vtmp[:], v_h[:, jt, :], dsum_all[:, 16 + jt:17 + jt])
                vt2 = vtmp[:].rearrange("p (c f) -> p c f", c=2)
                nc.vector.tensor_copy(v8[:, pr, :, par, :], vt2)
                nc.vector.tensor_tensor(
                    v8l[:, pr, :, par, :], vt2, v8[:, pr, :, par, :],
                    mybir.AluOpType.subtract,
                )
            else:
                nc.vector.tensor_scalar_mul(
                    v8[:, pr, :, par, :], vin,
                    dsum_all[:, 16 + jt:17 + jt])

        with tc.tile_pool(name="xload", bufs=1) as xp:
            xb = [xp.tile([P, 2, w], FP16, tag=f"xb{b}", name=f"xb{b}")
                  for b, (lo, w) in enumerate(XBLK)]
            # block0 right behind wk on the SP queue; block1 + biases on DVE q
            nc.sync.dma_start(
                xb[0][:], bass.AP(tensor=xp_t, offset=XOFF[0],
                                  ap=[[2 * N, P], [1, 2 * XBLK[0][1]]]))
            nc.scalar.dma_start(
                b_pack[:],
                bass.AP(tensor=bp_t, offset=0, ap=[[4 + C, P], [1, 4 + C]]))
            nc.scalar.dma_start(
                xb[1][:], bass.AP(tensor=xp_t, offset=XOFF[1],
                                  ap=[[2 * N, P], [1, 2 * XBLK[1][1]]]))
            nc.sync.dma_start(
                w_all[:, 2:4, :].rearrange("p w f -> p (w f)"),
                bass.AP(tensor=wp_t, offset=2 * C, ap=[[8 * C, P], [1, 2 * C]]),
            )
            nc.sync.dma_start(
                w_all[:, 4:6, :].rearrange("p w f -> p (w f)"),
                bass.AP(tensor=wp_t, offset=4 * C, ap=[[8 * C, P], [1, 2 * C]]),
            )
            for b, eng in ((2, nc.scalar), (3, nc.scalar), (4, nc.sync)):
                eng.dma_start(
                    xb[b][:], bass.AP(tensor=xp_t, offset=XOFF[b],
                                      ap=[[2 * N, P], [1, 2 * XBLK[b][1]]]))

            def xsl(ci, lo, size):
                for b, (blo, w) in enumerate(XBLK):
                    if blo <= lo and lo + size <= blo + w:
                        return xb[b][:, ci, lo - blo:lo - blo + size]
                raise AssertionError((lo, size))

            # ---- phase 1: k, q, v projections (fp16), early s_tiles ----
            early_s = [(jt0, iq0) for iq0 in range(3) for jt0 in range(4)]
            es_i = 0

            def emit_early_s(n, limit):
                nonlocal es_i
                while n > 0 and es_i < limit:
                    s_tile(*early_s[es_i])
                    es_i += 1
                    n -= 1

            with tc.tile_pool(name="ps_qkv", bufs=2, space="PSUM") as pq:
                for blk in range(4):
                    if blk < 2:
                        for ch in range(2):
                            pss = [pq.tile([P, 512], F32, tag="ps", name="ps") for _ in range(2)]
                            for ci in range(2):
                                lhs = wslice(1, ci, ch)
                                for t2 in range(2):
                                    t = blk * 2 + t2
                                    nc.tensor.matmul(
                                        pss[t2][:], lhs,
                                        xsl(ci, t * 512, 512),
                                        start=(ci == 0), stop=(ci == 1),
                                    )
                            for t2 in range(2):
                                t = blk * 2 + t2
                                nc.vector.tensor_scalar_add(
                                    k_h[ch][:, t * 512:(t + 1) * 512], pss[t2][:],
                                    b_all[:, 2 + ch:3 + ch])
                    for ch in range(2):
                        pss = [pq.tile([P, 512], F32, tag="ps", name="ps") for _ in range(2)]
                        for ci in range(2):
                            lhs = wslice(0, ci, ch)
                            for t2 in range(2):
                                ic = blk * 2 + t2
                                nc.tensor.matmul(
                                    pss[t2][:], lhs,
                                    xsl(ci, ic * 512, 512),
                                    start=(ci == 0), stop=(ci == 1),
                                )
                        for t2 in range(2):
                            ic = blk * 2 + t2
                            nc.vector.tensor_scalar_add(
                                q_h[ch][:, ic * 512:(ic + 1) * 512], pss[t2][:],
                                b_all[:, ch:ch + 1])
                        if blk >= 1:
                            emit_early_s(1, 4 * blk)
                        elif ch == 1:
                            emit_early_s(2, 2)
                    if blk < 2:
                        for jtg in range(2):
                            pss = [pq.tile([P, C], F32, tag="ps", name="ps") for _ in range(4)]
                            for ci in range(2):
                                for t in range(4):
                                    jt = blk * 8 + jtg * 4 + t
                                    nc.tensor.matmul(
                                        pss[t][:],
                                        xsl(ci, jt * P, P),
                                        w_all[:, 4 + ci, :],
                                        start=(ci == 0), stop=(ci == 1),
                                    )
                            for t in range(4):
                                nc.vector.tensor_add(
                                    v_h[:, blk * 8 + jtg * 4 + t, :], pss[t][:], bv_sb[:]
                                )
                            if blk == 1:
                                emit_early_s(1, 4 * blk)
                    if blk >= 2:
                        emit_early_s(2, 4 * blk)
                emit_early_s(len(early_s), len(early_s))  # remainder

        # ---- phase 2+3: scores/exp interleaved with fp8 DR out accum ---
        # groups: g0 = pairs 0..5 (jt 0..11), g1 = pairs 6..7 (jt 12..15)
        with tc.tile_pool(name="ps_o", bufs=2, space="PSUM") as po:

            def out_chain_pair(g, z):
                # two interleaved chains (z, ch=0) and (z, ch=1); z = i512 chunk
                prs = range(0, 6) if g == 0 else range(6, 8)
                ops = [po.tile([P, 512], F32, tag="og", name="og") for _ in range(2)]
                npr = len(prs)
                for si, pr in enumerate(prs):
                    for ch in range(2):
                        nc.tensor.matmul(
                            ops[ch][:], v8[:, pr, ch, :, :], attn8[:, pr, z, :, :],
                            start=(si == 0), stop=(si == npr - 1 and OUT_MODE != "dr2"),
                            perf_mode=DR,
                        )
                if OUT_MODE == "dr2":
                    for si, pr in enumerate(prs):
                        for ch in range(2):
                            nc.tensor.matmul(
                                ops[ch][:], v8l[:, pr, ch, :, :], attn8[:, pr, z, :, :],
                                start=False, stop=(si == npr - 1),
                                perf_mode=DR,
                            )
                for ch in range(2):
                    idx = z * 2 + ch
                    if g == 0:
                        nc.vector.tensor_copy(y_acc[:, idx, :], ops[ch][:])
                    else:
                        nc.vector.tensor_add(y_fin[:, ch, z, :], ops[ch][:],
                                             y_acc[:, idx, :])

            def y_store(n):
                # one DMA covers both ch halves x two z chunks (2KB/partition)
                eng = nc.sync if n % 2 == 0 else nc.scalar
                eng.dma_start(
                    bass.AP(tensor=y_t, offset=n * 1024,
                            ap=[[N, P], [P * N, 2], [1, 1024]]),
                    y_fin[:, :, 2 * n:2 * n + 2, :],
                )

            for jt in range(NJT):
                for iq in range(4):
                    if jt < 4 and iq < 3:
                        continue  # pre-emitted during the qkv phase
                    s_tile(jt, iq)
                finish_jt(jt)
                if jt >= 12:
                    z0 = (jt - 12) * 2
                    out_chain_pair(0, z0)
                    out_chain_pair(0, z0 + 1)
            for z in range(8):
                out_chain_pair(1, z)
                if z % 2 == 1:
                    y_store(z // 2)


_nc_cache = None
LAST_EXEC_TIME_NS = None
LAST_RESULT = None


def _get_nc():
    global _nc_cache
    if _nc_cache is None:
        _nc_cache = _build_module()
    return _nc_cache


def _pack_x(xb):
    # xb [C, N] fp16 -> [128, sum(2*w)] with per-block [p, ci, w] layout
    cols = []
    for lo, w in XBLK:
        blkv = xb[:, lo:lo + w].reshape(2, P, w).transpose(1, 0, 2)
        cols.append(np.ascontiguousarray(blkv).reshape(P, 2 * w))
    return np.ascontiguousarray(np.concatenate(cols, axis=1))


def kernel(x, wq, bq, wk, bk, wv, bv, wp, bp):
    global LAST_EXEC_TIME_NS, LAST_RESULT
    nc = _get_nc()

    f16 = np.float16
    x = np.asarray(x, dtype=np.float32).reshape(B, C, N).astype(f16)
    wq32 = np.asarray(wq, dtype=np.float32)
    wk32 = np.asarray(wk, dtype=np.float32)
    wv32 = np.asarray(wv, dtype=np.float32)
    wp32 = np.asarray(wp, dtype=np.float32)
    w2 = (wp32 @ wv32) * ALPHA            # fold output projection + ALPHA into v

    # w_pack slot order: wk(0,1), wq(2,3), w2(4,5), pad(6,7)
    wT = np.stack([wk32.T, wq32.T, w2.T]).astype(f16)    # [3, 256 ci, 256 co]
    w_pack = np.zeros((P, 8, C), dtype=f16)
    for w in range(3):
        w_pack[:, 2 * w:2 * w + 2, :] = wT[w].reshape(2, P, C).transpose(1, 0, 2)
    w_pack = np.ascontiguousarray(w_pack.reshape(P, 8 * C))

    bq32 = np.asarray(bq, dtype=np.float32)
    bk32 = np.asarray(bk, dtype=np.float32)
    b_pack = np.zeros((P, 4 + C), dtype=np.float32)
    b_pack[:, 0] = bq32[:P]
    b_pack[:, 1] = bq32[P:]
    b_pack[:, 2] = bk32[:P]
    b_pack[:, 3] = bk32[P:]
    b_pack[:, 4:] = (ALPHA * (wp32 @ np.asarray(bv, dtype=np.float32)))[None, :]
    b_pack = np.ascontiguousarray(b_pack)

    bp1 = np.asarray(bp, dtype=np.float32).reshape(C)

    in_maps = []
    for core in range(8):
        b, h = divmod(core, 2)
        xb = x[b] if h == 0 else np.ascontiguousarray(np.roll(x[b], -NJ, axis=1))
        in_maps.append({"xp": _pack_x(xb), "wp": w_pack, "bp": b_pack})

    res = bass_utils.run_bass_kernel_spmd(nc, in_maps, core_ids=list(range(8)))
    if res.exec_time_ns is not None:
        LAST_EXEC_TIME_NS = res.exec_time_ns
    LAST_RESULT = res

    y = np.zeros((B, C, N), np.float32)
    for b in range(B):
        ya = res.results[2 * b]["y"].astype(np.float32)
        yb = res.results[2 * b + 1]["y"].astype(np.float32)
        y[b] = ya + np.roll(yb, NJ, axis=1)
    y = y * (1.0 / ALPHA) + bp1.reshape(1, C, 1)
    return y.reshape(B, C, 64, 64)
